# revision 2
# baseline (speedup 1.0000x reference)
"""Swin-style windowed attention kernel for 8 TRN2 NeuronCores — v3.

Full inputs -> shard batch over 8 cores -> Bass/Tile kernel per core -> gather.

Per-core layout (hardcoded):
  4096 windows total, 512 windows/core, 49 tokens/window, dim 256, 8 heads x 32.
  Host pre-transposes x to xT [256, 25088] bf16 per core; softmax scale is
  folded into the q columns of w_qkv on host; the relative-position bias is
  preloaded into PSUM via identity matmuls and the dots accumulate onto it.

Flat software pipeline over 256 window pairs (2 windows on partition halves
{0-48, 64-112} via PE column tiling), with next-block xT DMA + qkv projection
groups interleaved between pair stages so the tensor engine never drains.
Per-head dots run concurrently on 3x2 PE sub-tiles (row = head base partition
0/32/64, col = window parity 0/64).
"""

import sys

sys.path.insert(0, "/opt/trn_rl_repo")

import numpy as np
import ml_dtypes

BF16 = ml_dtypes.bfloat16

DIM = 256
DH = 32
HEADS = 8
WIN = 7
N = WIN * WIN  # 49
SCALE = DIM ** -0.5
NCORES = 8
W_TOTAL = 16 * 16 * 16  # 4096 windows
W_CORE = W_TOTAL // NCORES  # 512
BW = 8  # windows per block
NB = W_CORE // BW  # 64 blocks
T = N * BW  # 392 tokens per block
TTOT = N * W_CORE  # 25088 tokens per core
NPAIR = W_CORE // 2  # 256 window pairs


# head h lives in projection group s = h // 3 at partition base 32 * (h % 3);
# dots PSUM bank r = h % 3, free slot s. Slot (r=2, s=2) is unused.
def _head_rs(h):
    return h % 3, h // 3


def _rel_pos_indices(window):
    pos = np.arange(window)
    gi, gj = np.meshgrid(pos, pos, indexing="ij")
    grid = np.stack([gi, gj], axis=-1).reshape(-1, 2)
    rel = grid[:, None, :] - grid[None, :, :] + (window - 1)
    return rel[..., 0] * (2 * window - 1) + rel[..., 1]


_PROG_CACHE = {}


def _build_program():
    import concourse.bass as bass
    import concourse.mybir as mybir
    from concourse import bacc
    from concourse.tile import TileContext

    f32 = mybir.dt.float32
    bf16 = mybir.dt.bfloat16

    nc = bacc.Bacc("TRN2", target_bir_lowering=False, debug=False, num_devices=NCORES)
    xt_d = nc.declare_dram_parameter("xt", [DIM, TTOT], bf16, isOutput=False)
    wqkv_d = nc.declare_dram_parameter("wqkv", [DIM, 3 * DIM], bf16, isOutput=False)
    wout_d = nc.declare_dram_parameter("wout", [DIM, DIM], bf16, isOutput=False)
    eb_d = nc.declare_dram_parameter("eb", [128, 3, 3 * N], bf16, isOutput=False)
    ident_d = nc.declare_dram_parameter("ident", [128, N], f32, isOutput=False)
    outt_d = nc.declare_dram_parameter("outt", [DIM, TTOT], bf16, isOutput=True)

    with TileContext(nc) as tc:
        with (
            tc.tile_pool(name="const", bufs=1) as cpool,
            tc.tile_pool(name="xt", bufs=3) as xpool,
            tc.tile_pool(name="qk", bufs=2) as qkpool,
            tc.tile_pool(name="vaug", bufs=2) as vpool,
            tc.tile_pool(name="et", bufs=2) as epool,
            tc.tile_pool(name="rec", bufs=2) as rpool,
            tc.tile_pool(name="on", bufs=2) as opool,
            tc.tile_pool(name="ot", bufs=2) as otpool,
            tc.tile_pool(name="outs", bufs=3) as outpool,
            tc.tile_pool(name="pbig", bufs=3, space="PSUM") as pbig,
            tc.tile_pool(name="pdps", bufs=1, space="PSUM") as pdps,
            tc.tile_pool(name="pavot", bufs=1, space="PSUM") as pavot,
        ):
            # --- constants loaded once ---
            wq_sb = cpool.tile([128, 2, 768], bf16, tag="wq")
            nc.sync.dma_start(out=wq_sb[:, 0, :], in_=wqkv_d[0:128, :])
            nc.sync.dma_start(out=wq_sb[:, 1, :], in_=wqkv_d[128:256, :])
            wo_sb = cpool.tile([128, 2, 256], bf16, tag="wo")
            nc.sync.dma_start(out=wo_sb[:, 0, :], in_=wout_d[0:128, :])
            nc.sync.dma_start(out=wo_sb[:, 1, :], in_=wout_d[128:256, :])
            eb_sb = cpool.tile([128, 3, 3 * N], bf16, tag="eb")
            nc.sync.dma_start(out=eb_sb[:], in_=eb_d[:])
            id_sb = cpool.tile([128, N], f32, tag="ident")
            nc.sync.dma_start(out=id_sb[:], in_=ident_d[:])

            # persistent PSUM tiles (single-buffered; memset once for sim)
            dps = pdps.tile([128, 3, 512], f32, tag="dps")
            nc.vector.memset(dps[:], 0.0)
            avot = pavot.tile([128, 2, 512], f32, tag="avot")
            nc.vector.memset(avot[:], 0.0)
            for _ in range(3):
                pz = pbig.tile([128, 512], f32, tag="big")
                nc.vector.memset(pz[:], 0.0)
            # vaug ones columns are constant: set once per rotating buffer
            for _ in range(2):
                vb = vpool.tile([128, HEADS, DH + 1], bf16, tag="vaug")
                nc.vector.memset(vb[:], 0.0)
                nc.vector.memset(vb[0:113, :, DH : DH + 1], 1.0)

            blkstate = {}

            def prologue_dma(b):
                off = T * b
                xt = xpool.tile([128, 2, T], bf16, tag="xt")
                nc.sync.dma_start(out=xt[:, 0, :], in_=xt_d[0:128, off : off + T])
                nc.sync.dma_start(out=xt[:, 1, :], in_=xt_d[128:256, off : off + T])
                qk_sb = qkpool.tile([128, 6, T], bf16, tag="qk")
                ot_sb = otpool.tile([128, 2, T], bf16, tag="ot")
                blkstate[b] = (xt, qk_sb, ot_sb)

            bounds = [(0, 96), (96, 192), (192, 256)]

            def proj_groups(b, gs):
                xt, qk_sb, _ = blkstate[b]
                for g in gs:
                    lo, hi = bounds[g % 3]
                    base = 0 if g < 3 else 256
                    m = hi - lo
                    ps = pbig.tile([128, 512], f32, tag="big")
                    for kc in range(2):
                        nc.tensor.matmul(
                            ps[0:m, 0:T],
                            lhsT=wq_sb[:, kc, base + lo : base + hi],
                            rhs=xt[:, kc, :],
                            start=(kc == 0),
                            stop=(kc == 1),
                        )
                    if g % 2 == 0:
                        nc.scalar.copy(qk_sb[0:m, g, :], ps[0:m, 0:T])
                    else:
                        nc.vector.tensor_copy(qk_sb[0:m, g, :], ps[0:m, 0:T])

            st = {}

            def stage_a(q):
                b, p = q // 4, q % 4
                xt, qk_sb, _ = blkstate[b]
                c0 = 98 * p
                # V projection for the pair (col-tiled by parity)
                vps = pbig.tile([128, 512], f32, tag="big")
                for par in range(2):
                    cw = c0 + 49 * par
                    for kc in range(2):
                        nc.tensor.matmul(
                            vps[64 * par : 64 * par + 49, 0:256],
                            lhsT=xt[:, kc, cw : cw + 49],
                            rhs=wq_sb[:, kc, 512:768],
                            start=(kc == 0),
                            stop=(kc == 1),
                        )
                # dots: 16 MMs over 3x2 sub-tiles
                for par in range(2):
                    jb = 64 * par
                    cw = c0 + 49 * par
                    for h in range(HEADS):
                        r, s = _head_rs(h)
                        bp = 32 * r
                        nc.tensor.matmul(
                            dps[jb : jb + 49, r, 49 * s : 49 * s + 49],
                            lhsT=qk_sb[bp : bp + 32, 3 + s, cw : cw + 49],
                            rhs=qk_sb[bp : bp + 32, s, cw : cw + 49],
                            start=True,
                            stop=True,
                        )
                vaug = vpool.tile([128, HEADS, DH + 1], bf16, tag="vaug")
                nc.scalar.copy(
                    vaug[0:113, :, 0:DH],
                    vps[0:113, 0:256].rearrange("p (h d) -> p h d", h=HEADS),
                )
                # exp over both windows, all heads (cross-bank AP)
                et_raw = epool.tile([128, 3, 3 * N], bf16, tag="etraw")
                nc.scalar.activation(
                    out=et_raw[0:113, :, :],
                    in_=dps[0:113, :, 0 : 3 * N],
                    func=mybir.ActivationFunctionType.Exp,
                )
                et = epool.tile([128, 3, 3 * N], bf16, tag="et")
                nc.vector.tensor_tensor(
                    out=et[0:113, :, :],
                    in0=et_raw[0:113, :, :],
                    in1=eb_sb[0:113, :, :],
                    op=mybir.AluOpType.mult,
                )
                st[q] = (vaug, et)

            def stage_b1(q):
                vaug, et = st[q]
                for par in range(2):
                    jb = 64 * par
                    for h in range(HEADS):
                        r, s = _head_rs(h)
                        nc.tensor.matmul(
                            avot[jb : jb + 49, par, 33 * h : 33 * h + 33],
                            lhsT=et[jb : jb + 49, r, 49 * s : 49 * s + 49],
                            rhs=vaug[jb : jb + 49, h, :],
                            start=True,
                            stop=True,
                        )
                rec = rpool.tile([128, HEADS], f32, tag="rec")
                o_n = opool.tile([128, HEADS, DH], f32, tag="on")
                for par in range(2):
                    pl, ph = 64 * par, 64 * par + 49
                    aview = avot[pl:ph, par, 0:264].rearrange(
                        "p (h e) -> p h e", h=HEADS
                    )
                    nc.vector.reciprocal(
                        out=rec[pl:ph, :].unsqueeze(2),
                        in_=aview[:, :, DH : DH + 1],
                    )
                    nc.vector.tensor_tensor(
                        out=o_n[pl:ph, :, :],
                        in0=aview[:, :, 0:DH],
                        in1=rec[pl:ph, :].unsqueeze(2).broadcast_to([49, HEADS, DH]),
                        op=mybir.AluOpType.mult,
                    )
                st[q] = (vaug, et, o_n)

            def stage_b2(q):
                b, p = q // 4, q % 4
                ot_sb = blkstate[b][2]
                c0 = 98 * p
                o_n = st.pop(q)[2]
                for par in range(2):
                    pl, ph = 64 * par, 64 * par + 49
                    for kc in range(2):
                        nc.tensor.transpose(
                            avot[0:128, par, 264 + 49 * kc : 313 + 49 * kc],
                            o_n[pl:ph, 4 * kc : 4 * kc + 4, :].rearrange(
                                "p a b -> p (a b)"
                            ),
                            id_sb[pl:ph, :],
                        )
                nc.vector.tensor_copy(
                    ot_sb[0:128, :, c0 : c0 + 98].rearrange(
                        "p k (w c) -> p k w c", w=2
                    ),
                    avot[0:128, :, 264:362].rearrange("p a (b c) -> p b a c", b=2),
                )

            def outproj(b):
                off = T * b
                ot_sb = blkstate.pop(b)[2]
                for mc in range(2):
                    pps = pbig.tile([128, 512], f32, tag="big")
                    for kc in range(2):
                        nc.tensor.matmul(
                            pps[:, 0:T],
                            lhsT=wo_sb[:, kc, 128 * mc : 128 * (mc + 1)],
                            rhs=ot_sb[:, kc, :],
                            start=(kc == 0),
                            stop=(kc == 1),
                        )
                    os_sb = outpool.tile([128, T], bf16, tag="outs")
                    if mc == 0:
                        nc.scalar.copy(os_sb[:], pps[:, 0:T])
                    else:
                        nc.vector.tensor_copy(os_sb[:], pps[:, 0:T])
                    nc.sync.dma_start(
                        out=outt_d[128 * mc : 128 * (mc + 1), off : off + T],
                        in_=os_sb[:],
                    )

            # --- flat software-pipelined schedule over all pairs ---
            prologue_dma(0)
            proj_groups(0, [0, 1, 2, 3, 4, 5])
            stage_a(0)
            stage_a(1)
            for q in range(NPAIR):
                b, p = q // 4, q % 4
                if p == 0 and b + 1 < NB:
                    prologue_dma(b + 1)
                stage_b1(q)
                if p in (1, 2) and b + 1 < NB:
                    proj_groups(b + 1, [0, 1, 2] if p == 1 else [3, 4, 5])
                if q + 2 < NPAIR:
                    stage_a(q + 2)
                stage_b2(q)
                if p == 3:
                    outproj(b)
    nc.compile()
    return nc


def _host_prep(w_qkv, w_out, bias_table):
    wqkv = np.asarray(w_qkv, dtype=np.float32).copy()
    wqkv[:, 0:DIM] *= SCALE  # fold softmax scale into q projection
    wqkv_b = np.ascontiguousarray(wqkv).astype(BF16)
    wout_b = np.ascontiguousarray(np.asarray(w_out, dtype=np.float32)).astype(BF16)

    rel = _rel_pos_indices(WIN)  # [i, j]
    bias = np.asarray(bias_table, dtype=np.float32)[rel]  # [i, j, h]
    bt = np.ones((128, 3, 3 * N), dtype=np.float32)
    for h in range(HEADS):
        r, s = _head_rs(h)
        blk = np.exp(bias[:, :, h].T)  # [j, i]
        for par in range(2):
            bt[64 * par : 64 * par + 49, r, 49 * s : 49 * s + 49] = blk
    bias_b = bt.astype(BF16)

    ident = np.zeros((128, N), dtype=np.float32)
    ident[0:49] = np.eye(N, dtype=np.float32)
    ident[64:113] = np.eye(N, dtype=np.float32)
    return wqkv_b, wout_b, bias_b, ident


def make_in_maps(x, w_qkv, w_out, bias_table):
    wqkv_b, wout_b, bias_b, ident = _host_prep(w_qkv, w_out, bias_table)
    xf = np.asarray(x, dtype=np.float32).reshape(W_TOTAL, N, DIM)
    in_maps = []
    for c in range(NCORES):
        xs = xf[c * W_CORE : (c + 1) * W_CORE].reshape(TTOT, DIM)
        xs_t = np.ascontiguousarray(xs.T).astype(BF16)
        in_maps.append(
            {
                "xt": xs_t,
                "wqkv": wqkv_b,
                "wout": wout_b,
                "eb": bias_b,
                "ident": ident,
            }
        )
    return in_maps


def kernel(x, w_qkv, w_out, bias_table):
    if "nc" not in _PROG_CACHE:
        _PROG_CACHE["nc"] = _build_program()
    nc = _PROG_CACHE["nc"]

    from concourse.bass_utils import run_bass_kernel_spmd

    in_maps = make_in_maps(x, w_qkv, w_out, bias_table)
    res = run_bass_kernel_spmd(nc, in_maps, list(range(NCORES)))
    outs = []
    for c in range(NCORES):
        ot = np.asarray(res.results[c]["outt"]).astype(np.float32)  # [256, TTOT]
        outs.append(ot.T.reshape(W_CORE, N, DIM))
    full = np.concatenate(outs, axis=0)
    return full.reshape(16, 16, 16, WIN, WIN, DIM).astype(np.float32)


# revision 3
# speedup vs baseline: 1.1986x; 1.1986x over previous
"""Swin-style windowed attention kernel for 8 TRN2 NeuronCores — v3.

Full inputs -> shard batch over 8 cores -> Bass/Tile kernel per core -> gather.

Per-core layout (hardcoded):
  4096 windows total, 512 windows/core, 49 tokens/window, dim 256, 8 heads x 32.
  Host pre-transposes x to xT [256, 25088] bf16 per core; softmax scale is
  folded into the q columns of w_qkv on host; the relative-position bias is
  preloaded into PSUM via identity matmuls and the dots accumulate onto it.

Flat software pipeline over 256 window pairs (2 windows on partition halves
{0-48, 64-112} via PE column tiling), with next-block xT DMA + qkv projection
groups interleaved between pair stages so the tensor engine never drains.
Per-head dots run concurrently on 3x2 PE sub-tiles (row = head base partition
0/32/64, col = window parity 0/64).
"""

import sys

sys.path.insert(0, "/opt/trn_rl_repo")

import numpy as np
import ml_dtypes

BF16 = ml_dtypes.bfloat16

DIM = 256
DH = 32
HEADS = 8
WIN = 7
N = WIN * WIN  # 49
SCALE = DIM ** -0.5
NCORES = 8
W_TOTAL = 16 * 16 * 16  # 4096 windows
W_CORE = W_TOTAL // NCORES  # 512
BW = 8  # windows per block
NB = W_CORE // BW  # 64 blocks
T = N * BW  # 392 tokens per block
TTOT = N * W_CORE  # 25088 tokens per core
NPAIR = W_CORE // 2  # 256 window pairs


# head h lives in projection group s = h // 3 at partition base 32 * (h % 3);
# dots PSUM bank r = h % 3, free slot s. Slot (r=2, s=2) is unused.
def _head_rs(h):
    return h % 3, h // 3


def _rel_pos_indices(window):
    pos = np.arange(window)
    gi, gj = np.meshgrid(pos, pos, indexing="ij")
    grid = np.stack([gi, gj], axis=-1).reshape(-1, 2)
    rel = grid[:, None, :] - grid[None, :, :] + (window - 1)
    return rel[..., 0] * (2 * window - 1) + rel[..., 1]


_PROG_CACHE = {}


def _build_program():
    import concourse.bass as bass
    import concourse.mybir as mybir
    from concourse import bacc
    from concourse.tile import TileContext

    f32 = mybir.dt.float32
    bf16 = mybir.dt.bfloat16

    nc = bacc.Bacc("TRN2", target_bir_lowering=False, debug=False, num_devices=NCORES)
    xt_d = nc.declare_dram_parameter("xt", [DIM, TTOT], bf16, isOutput=False)
    wqkv_d = nc.declare_dram_parameter("wqkv", [DIM, 3 * DIM], bf16, isOutput=False)
    wout_d = nc.declare_dram_parameter("wout", [DIM, DIM], bf16, isOutput=False)
    eb_d = nc.declare_dram_parameter("eb", [128, 3, 3 * N], bf16, isOutput=False)
    ident_d = nc.declare_dram_parameter("ident", [128, N], f32, isOutput=False)
    outt_d = nc.declare_dram_parameter("outt", [DIM, TTOT], bf16, isOutput=True)

    with TileContext(nc) as tc:
        with (
            tc.tile_pool(name="const", bufs=1) as cpool,
            tc.tile_pool(name="xt", bufs=3) as xpool,
            tc.tile_pool(name="qk", bufs=2) as qkpool,
            tc.tile_pool(name="vaug", bufs=2) as vpool,
            tc.tile_pool(name="et", bufs=2) as epool,
            tc.tile_pool(name="rec", bufs=2) as rpool,
            tc.tile_pool(name="on", bufs=2) as opool,
            tc.tile_pool(name="ot", bufs=2) as otpool,
            tc.tile_pool(name="outs", bufs=3) as outpool,
            tc.tile_pool(name="pbig", bufs=3, space="PSUM") as pbig,
            tc.tile_pool(name="pdps", bufs=1, space="PSUM") as pdps,
            tc.tile_pool(name="pavot", bufs=1, space="PSUM") as pavot,
        ):
            # --- constants loaded once ---
            wq_sb = cpool.tile([128, 2, 768], bf16, tag="wq")
            nc.sync.dma_start(out=wq_sb[:, 0, :], in_=wqkv_d[0:128, :])
            nc.sync.dma_start(out=wq_sb[:, 1, :], in_=wqkv_d[128:256, :])
            wo_sb = cpool.tile([128, 2, 256], bf16, tag="wo")
            nc.sync.dma_start(out=wo_sb[:, 0, :], in_=wout_d[0:128, :])
            nc.sync.dma_start(out=wo_sb[:, 1, :], in_=wout_d[128:256, :])
            eb_sb = cpool.tile([128, 3, 3 * N], bf16, tag="eb")
            nc.sync.dma_start(out=eb_sb[:], in_=eb_d[:])
            id_sb = cpool.tile([128, N], f32, tag="ident")
            nc.sync.dma_start(out=id_sb[:], in_=ident_d[:])

            # persistent PSUM tiles (single-buffered; memset once for sim)
            dps = pdps.tile([128, 3, 512], f32, tag="dps")
            nc.vector.memset(dps[:], 0.0)
            avot = pavot.tile([128, 2, 512], f32, tag="avot")
            nc.vector.memset(avot[:], 0.0)
            for _ in range(3):
                pz = pbig.tile([128, 512], f32, tag="big")
                nc.vector.memset(pz[:], 0.0)
            # vaug ones columns are constant: set once per rotating buffer
            for _ in range(2):
                vb = vpool.tile([128, HEADS, DH + 1], bf16, tag="vaug")
                nc.vector.memset(vb[:], 0.0)
                nc.vector.memset(vb[0:113, :, DH : DH + 1], 1.0)

            blkstate = {}

            def prologue_dma(b):
                off = T * b
                xt = xpool.tile([128, 2, T], bf16, tag="xt")
                nc.sync.dma_start(out=xt[:, 0, :], in_=xt_d[0:128, off : off + T])
                nc.sync.dma_start(out=xt[:, 1, :], in_=xt_d[128:256, off : off + T])
                qk_sb = qkpool.tile([128, 6, T], bf16, tag="qk")
                ot_sb = otpool.tile([128, 2, T], bf16, tag="ot")
                blkstate[b] = (xt, qk_sb, ot_sb)

            bounds = [(0, 96), (96, 192), (192, 256)]

            def proj_groups(b, gs):
                xt, qk_sb, _ = blkstate[b]
                for g in gs:
                    lo, hi = bounds[g % 3]
                    base = 0 if g < 3 else 256
                    m = hi - lo
                    ps = pbig.tile([128, 512], f32, tag="big")
                    for kc in range(2):
                        nc.tensor.matmul(
                            ps[0:m, 0:T],
                            lhsT=wq_sb[:, kc, base + lo : base + hi],
                            rhs=xt[:, kc, :],
                            start=(kc == 0),
                            stop=(kc == 1),
                        )
                    if g % 2 == 0:
                        nc.scalar.copy(qk_sb[0:m, g, :], ps[0:m, 0:T])
                    else:
                        nc.vector.tensor_copy(qk_sb[0:m, g, :], ps[0:m, 0:T])

            st = {}

            def stage_a(q):
                b, p = q // 4, q % 4
                xt, qk_sb, _ = blkstate[b]
                c0 = 98 * p
                # V projection for the pair (col-tiled by parity)
                vps = pbig.tile([128, 512], f32, tag="big")
                for par in range(2):
                    cw = c0 + 49 * par
                    for kc in range(2):
                        nc.tensor.matmul(
                            vps[64 * par : 64 * par + 49, 0:256],
                            lhsT=xt[:, kc, cw : cw + 49],
                            rhs=wq_sb[:, kc, 512:768],
                            start=(kc == 0),
                            stop=(kc == 1),
                        )
                # dots: 16 MMs over 3x2 sub-tiles
                for par in range(2):
                    jb = 64 * par
                    cw = c0 + 49 * par
                    for h in range(HEADS):
                        r, s = _head_rs(h)
                        bp = 32 * r
                        nc.tensor.matmul(
                            dps[jb : jb + 49, r, 49 * s : 49 * s + 49],
                            lhsT=qk_sb[bp : bp + 32, 3 + s, cw : cw + 49],
                            rhs=qk_sb[bp : bp + 32, s, cw : cw + 49],
                            start=True,
                            stop=True,
                        )
                vaug = vpool.tile([128, HEADS, DH + 1], bf16, tag="vaug")
                nc.scalar.copy(
                    vaug[0:113, :, 0:DH],
                    vps[0:113, 0:256].rearrange("p (h d) -> p h d", h=HEADS),
                )
                # exp over both windows, all heads (cross-bank AP)
                et_raw = epool.tile([128, 3, 3 * N], bf16, tag="etraw")
                nc.scalar.activation(
                    out=et_raw[0:113, :, :],
                    in_=dps[0:113, :, 0 : 3 * N],
                    func=mybir.ActivationFunctionType.Exp,
                )
                st[q] = (vaug, et_raw)

            def stage_b1(q):
                vaug, et_raw = st[q]
                et = epool.tile([128, 3, 3 * N], bf16, tag="et")
                nc.vector.tensor_tensor(
                    out=et[0:113, :, :],
                    in0=et_raw[0:113, :, :],
                    in1=eb_sb[0:113, :, :],
                    op=mybir.AluOpType.mult,
                )
                for par in range(2):
                    jb = 64 * par
                    for h in range(HEADS):
                        r, s = _head_rs(h)
                        nc.tensor.matmul(
                            avot[jb : jb + 49, par, 33 * h : 33 * h + 33],
                            lhsT=et[jb : jb + 49, r, 49 * s : 49 * s + 49],
                            rhs=vaug[jb : jb + 49, h, :],
                            start=True,
                            stop=True,
                        )
                rec = rpool.tile([128, HEADS], f32, tag="rec")
                o_n = opool.tile([128, HEADS, DH], f32, tag="on")
                for par in range(2):
                    pl, ph = 64 * par, 64 * par + 49
                    aview = avot[pl:ph, par, 0:264].rearrange(
                        "p (h e) -> p h e", h=HEADS
                    )
                    nc.vector.reciprocal(
                        out=rec[pl:ph, :].unsqueeze(2),
                        in_=aview[:, :, DH : DH + 1],
                    )
                    nc.vector.tensor_tensor(
                        out=o_n[pl:ph, :, :],
                        in0=aview[:, :, 0:DH],
                        in1=rec[pl:ph, :].unsqueeze(2).broadcast_to([49, HEADS, DH]),
                        op=mybir.AluOpType.mult,
                    )
                st[q] = (vaug, et, o_n)

            def stage_b2(q):
                b, p = q // 4, q % 4
                ot_sb = blkstate[b][2]
                c0 = 98 * p
                o_n = st.pop(q)[2]
                for par in range(2):
                    pl, ph = 64 * par, 64 * par + 49
                    for kc in range(2):
                        nc.tensor.transpose(
                            avot[0:128, par, 264 + 49 * kc : 313 + 49 * kc],
                            o_n[pl:ph, 4 * kc : 4 * kc + 4, :].rearrange(
                                "p a b -> p (a b)"
                            ),
                            id_sb[pl:ph, :],
                        )
                nc.vector.tensor_copy(
                    ot_sb[0:128, :, c0 : c0 + 98].rearrange(
                        "p k (w c) -> p k w c", w=2
                    ),
                    avot[0:128, :, 264:362].rearrange("p a (b c) -> p b a c", b=2),
                )

            def outproj(b):
                off = T * b
                ot_sb = blkstate.pop(b)[2]
                for mc in range(2):
                    pps = pbig.tile([128, 512], f32, tag="big")
                    for kc in range(2):
                        nc.tensor.matmul(
                            pps[:, 0:T],
                            lhsT=wo_sb[:, kc, 128 * mc : 128 * (mc + 1)],
                            rhs=ot_sb[:, kc, :],
                            start=(kc == 0),
                            stop=(kc == 1),
                        )
                    os_sb = outpool.tile([128, T], bf16, tag="outs")
                    if mc == 0:
                        nc.scalar.copy(os_sb[:], pps[:, 0:T])
                    else:
                        nc.vector.tensor_copy(os_sb[:], pps[:, 0:T])
                    nc.sync.dma_start(
                        out=outt_d[128 * mc : 128 * (mc + 1), off : off + T],
                        in_=os_sb[:],
                    )

            # --- flat software-pipelined schedule over all pairs ---
            prologue_dma(0)
            proj_groups(0, [0, 1, 2, 3, 4, 5])
            stage_a(0)
            stage_a(1)
            for q in range(NPAIR):
                b, p = q // 4, q % 4
                if p == 0 and b + 1 < NB:
                    prologue_dma(b + 1)
                stage_b1(q)
                if p in (1, 2) and b + 1 < NB:
                    proj_groups(b + 1, [0, 1, 2] if p == 1 else [3, 4, 5])
                if q + 2 < NPAIR:
                    stage_a(q + 2)
                stage_b2(q)
                if p == 3:
                    outproj(b)
    nc.compile()
    return nc


def _host_prep(w_qkv, w_out, bias_table):
    wqkv = np.asarray(w_qkv, dtype=np.float32).copy()
    wqkv[:, 0:DIM] *= SCALE  # fold softmax scale into q projection
    wqkv_b = np.ascontiguousarray(wqkv).astype(BF16)
    wout_b = np.ascontiguousarray(np.asarray(w_out, dtype=np.float32)).astype(BF16)

    rel = _rel_pos_indices(WIN)  # [i, j]
    bias = np.asarray(bias_table, dtype=np.float32)[rel]  # [i, j, h]
    bt = np.ones((128, 3, 3 * N), dtype=np.float32)
    for h in range(HEADS):
        r, s = _head_rs(h)
        blk = np.exp(bias[:, :, h].T)  # [j, i]
        for par in range(2):
            bt[64 * par : 64 * par + 49, r, 49 * s : 49 * s + 49] = blk
    bias_b = bt.astype(BF16)

    ident = np.zeros((128, N), dtype=np.float32)
    ident[0:49] = np.eye(N, dtype=np.float32)
    ident[64:113] = np.eye(N, dtype=np.float32)
    return wqkv_b, wout_b, bias_b, ident


def make_in_maps(x, w_qkv, w_out, bias_table):
    wqkv_b, wout_b, bias_b, ident = _host_prep(w_qkv, w_out, bias_table)
    xf = np.asarray(x, dtype=np.float32).reshape(W_TOTAL, N, DIM)
    in_maps = []
    for c in range(NCORES):
        xs = xf[c * W_CORE : (c + 1) * W_CORE].reshape(TTOT, DIM)
        xs_t = np.ascontiguousarray(xs.T).astype(BF16)
        in_maps.append(
            {
                "xt": xs_t,
                "wqkv": wqkv_b,
                "wout": wout_b,
                "eb": bias_b,
                "ident": ident,
            }
        )
    return in_maps


def kernel(x, w_qkv, w_out, bias_table):
    if "nc" not in _PROG_CACHE:
        _PROG_CACHE["nc"] = _build_program()
    nc = _PROG_CACHE["nc"]

    from concourse.bass_utils import run_bass_kernel_spmd

    in_maps = make_in_maps(x, w_qkv, w_out, bias_table)
    res = run_bass_kernel_spmd(nc, in_maps, list(range(NCORES)))
    outs = []
    for c in range(NCORES):
        ot = np.asarray(res.results[c]["outt"]).astype(np.float32)  # [256, TTOT]
        outs.append(ot.T.reshape(W_CORE, N, DIM))
    full = np.concatenate(outs, axis=0)
    return full.reshape(16, 16, 16, WIN, WIN, DIM).astype(np.float32)


# revision 4
# speedup vs baseline: 1.2205x; 1.0183x over previous
"""Swin-style windowed attention kernel for 8 TRN2 NeuronCores — v3.

Full inputs -> shard batch over 8 cores -> Bass/Tile kernel per core -> gather.

Per-core layout (hardcoded):
  4096 windows total, 512 windows/core, 49 tokens/window, dim 256, 8 heads x 32.
  Host pre-transposes x to xT [256, 25088] bf16 per core; softmax scale is
  folded into the q columns of w_qkv on host; the relative-position bias is
  preloaded into PSUM via identity matmuls and the dots accumulate onto it.

Flat software pipeline over 256 window pairs (2 windows on partition halves
{0-48, 64-112} via PE column tiling), with next-block xT DMA + qkv projection
groups interleaved between pair stages so the tensor engine never drains.
Per-head dots run concurrently on 3x2 PE sub-tiles (row = head base partition
0/32/64, col = window parity 0/64).
"""

import sys

sys.path.insert(0, "/opt/trn_rl_repo")

import numpy as np
import ml_dtypes

BF16 = ml_dtypes.bfloat16

DIM = 256
DH = 32
HEADS = 8
WIN = 7
N = WIN * WIN  # 49
SCALE = DIM ** -0.5
NCORES = 8
W_TOTAL = 16 * 16 * 16  # 4096 windows
W_CORE = W_TOTAL // NCORES  # 512
BW = 8  # windows per block
NB = W_CORE // BW  # 64 blocks
T = N * BW  # 392 tokens per block
TTOT = N * W_CORE  # 25088 tokens per core
NPAIR = W_CORE // 2  # 256 window pairs


# head h lives in projection group s = h // 3 at partition base 32 * (h % 3);
# dots PSUM bank r = h % 3, free slot s. Slot (r=2, s=2) is unused.
def _head_rs(h):
    return h % 3, h // 3


def _rel_pos_indices(window):
    pos = np.arange(window)
    gi, gj = np.meshgrid(pos, pos, indexing="ij")
    grid = np.stack([gi, gj], axis=-1).reshape(-1, 2)
    rel = grid[:, None, :] - grid[None, :, :] + (window - 1)
    return rel[..., 0] * (2 * window - 1) + rel[..., 1]


_PROG_CACHE = {}


def _build_program():
    import concourse.bass as bass
    import concourse.mybir as mybir
    from concourse import bacc
    from concourse.tile import TileContext

    f32 = mybir.dt.float32
    bf16 = mybir.dt.bfloat16

    nc = bacc.Bacc("TRN2", target_bir_lowering=False, debug=False, num_devices=NCORES)
    xt_d = nc.declare_dram_parameter("xt", [DIM, TTOT], bf16, isOutput=False)
    wqkv_d = nc.declare_dram_parameter("wqkv", [DIM, 3 * DIM], bf16, isOutput=False)
    wout_d = nc.declare_dram_parameter("wout", [DIM, DIM], bf16, isOutput=False)
    eb_d = nc.declare_dram_parameter("eb", [128, 3, 3 * N], bf16, isOutput=False)
    ident_d = nc.declare_dram_parameter("ident", [128, N], f32, isOutput=False)
    outt_d = nc.declare_dram_parameter("outt", [DIM, TTOT], bf16, isOutput=True)

    with TileContext(nc) as tc:
        with (
            tc.tile_pool(name="const", bufs=1) as cpool,
            tc.tile_pool(name="xt", bufs=3) as xpool,
            tc.tile_pool(name="qk", bufs=2) as qkpool,
            tc.tile_pool(name="vaug", bufs=2) as vpool,
            tc.tile_pool(name="et", bufs=2) as epool,
            tc.tile_pool(name="rec", bufs=2) as rpool,
            tc.tile_pool(name="on", bufs=2) as opool,
            tc.tile_pool(name="ot", bufs=2) as otpool,
            tc.tile_pool(name="outs", bufs=3) as outpool,
            tc.tile_pool(name="pbig", bufs=3, space="PSUM") as pbig,
            tc.tile_pool(name="pdps", bufs=1, space="PSUM") as pdps,
            tc.tile_pool(name="pavot", bufs=1, space="PSUM") as pavot,
        ):
            # --- constants loaded once ---
            wq_sb = cpool.tile([128, 2, 768], bf16, tag="wq")
            nc.sync.dma_start(out=wq_sb[:, 0, :], in_=wqkv_d[0:128, :])
            nc.sync.dma_start(out=wq_sb[:, 1, :], in_=wqkv_d[128:256, :])
            wo_sb = cpool.tile([128, 2, 256], bf16, tag="wo")
            nc.sync.dma_start(out=wo_sb[:, 0, :], in_=wout_d[0:128, :])
            nc.sync.dma_start(out=wo_sb[:, 1, :], in_=wout_d[128:256, :])
            eb_sb = cpool.tile([128, 3, 3 * N], bf16, tag="eb")
            nc.sync.dma_start(out=eb_sb[:], in_=eb_d[:])
            id_sb = cpool.tile([128, N], f32, tag="ident")
            nc.sync.dma_start(out=id_sb[:], in_=ident_d[:])

            # persistent PSUM tiles (single-buffered; memset once for sim)
            dps = pdps.tile([128, 3, 512], f32, tag="dps")
            nc.vector.memset(dps[:], 0.0)
            avot = pavot.tile([128, 2, 512], f32, tag="avot")
            nc.vector.memset(avot[:], 0.0)
            for _ in range(3):
                pz = pbig.tile([128, 512], f32, tag="big")
                nc.vector.memset(pz[:], 0.0)
            # vaug ones columns are constant: set once per rotating buffer
            for _ in range(2):
                vb = vpool.tile([128, HEADS, DH + 1], bf16, tag="vaug")
                nc.vector.memset(vb[:], 0.0)
                nc.vector.memset(vb[0:113, :, DH : DH + 1], 1.0)

            blkstate = {}

            def prologue_dma(b):
                off = T * b
                xt = xpool.tile([128, 2, T], bf16, tag="xt")
                nc.sync.dma_start(out=xt[:, 0, :], in_=xt_d[0:128, off : off + T])
                nc.sync.dma_start(out=xt[:, 1, :], in_=xt_d[128:256, off : off + T])
                qk_sb = qkpool.tile([128, 6, T], bf16, tag="qk")
                ot_sb = otpool.tile([128, 2, T], bf16, tag="ot")
                blkstate[b] = (xt, qk_sb, ot_sb)

            bounds = [(0, 96), (96, 192), (192, 256)]

            def proj_groups(b, gs):
                xt, qk_sb, _ = blkstate[b]
                for g in gs:
                    lo, hi = bounds[g % 3]
                    base = 0 if g < 3 else 256
                    m = hi - lo
                    ps = pbig.tile([128, 512], f32, tag="big")
                    for kc in range(2):
                        nc.tensor.matmul(
                            ps[0:m, 0:T],
                            lhsT=wq_sb[:, kc, base + lo : base + hi],
                            rhs=xt[:, kc, :],
                            start=(kc == 0),
                            stop=(kc == 1),
                        )
                    if g % 2 == 0:
                        nc.scalar.copy(qk_sb[0:m, g, :], ps[0:m, 0:T])
                    else:
                        nc.vector.tensor_copy(qk_sb[0:m, g, :], ps[0:m, 0:T])

            st = {}

            def stage_a(q):
                b, p = q // 4, q % 4
                xt, qk_sb, _ = blkstate[b]
                c0 = 98 * p
                # V projection for the pair (col-tiled by parity)
                vps = pbig.tile([128, 512], f32, tag="big")
                for par in range(2):
                    cw = c0 + 49 * par
                    for kc in range(2):
                        nc.tensor.matmul(
                            vps[64 * par : 64 * par + 49, 0:256],
                            lhsT=xt[:, kc, cw : cw + 49],
                            rhs=wq_sb[:, kc, 512:768],
                            start=(kc == 0),
                            stop=(kc == 1),
                        )
                # dots: 16 MMs over 3x2 sub-tiles
                for par in range(2):
                    jb = 64 * par
                    cw = c0 + 49 * par
                    for h in range(HEADS):
                        r, s = _head_rs(h)
                        bp = 32 * r
                        nc.tensor.matmul(
                            dps[jb : jb + 49, r, 49 * s : 49 * s + 49],
                            lhsT=qk_sb[bp : bp + 32, 3 + s, cw : cw + 49],
                            rhs=qk_sb[bp : bp + 32, s, cw : cw + 49],
                            start=True,
                            stop=True,
                        )
                vaug = vpool.tile([128, HEADS, DH + 1], bf16, tag="vaug")
                nc.scalar.copy(
                    vaug[0:113, :, 0:DH],
                    vps[0:113, 0:256].rearrange("p (h d) -> p h d", h=HEADS),
                )
                # exp over both windows, all heads (cross-bank AP)
                et_raw = epool.tile([128, 3, 3 * N], bf16, tag="etraw")
                nc.scalar.activation(
                    out=et_raw[0:113, :, :],
                    in_=dps[0:113, :, 0 : 3 * N],
                    func=mybir.ActivationFunctionType.Exp,
                )
                st[q] = (vaug, et_raw)

            def stage_b1(q):
                vaug, et_raw = st[q]
                et = epool.tile([128, 3, 3 * N], bf16, tag="et")
                nc.vector.tensor_tensor(
                    out=et[0:113, :, :],
                    in0=et_raw[0:113, :, :],
                    in1=eb_sb[0:113, :, :],
                    op=mybir.AluOpType.mult,
                )
                for par in range(2):
                    jb = 64 * par
                    for h in range(HEADS):
                        r, s = _head_rs(h)
                        nc.tensor.matmul(
                            avot[jb : jb + 49, par, 33 * h : 33 * h + 33],
                            lhsT=et[jb : jb + 49, r, 49 * s : 49 * s + 49],
                            rhs=vaug[jb : jb + 49, h, :],
                            start=True,
                            stop=True,
                        )
                rec = rpool.tile([128, HEADS], f32, tag="rec")
                o_n = opool.tile([128, HEADS, DH], f32, tag="on")
                for par in range(2):
                    pl, ph = 64 * par, 64 * par + 49
                    aview = avot[pl:ph, par, 0:264].rearrange(
                        "p (h e) -> p h e", h=HEADS
                    )
                    nc.vector.reciprocal(
                        out=rec[pl:ph, :].unsqueeze(2),
                        in_=aview[:, :, DH : DH + 1],
                    )
                    nc.vector.tensor_tensor(
                        out=o_n[pl:ph, :, :],
                        in0=aview[:, :, 0:DH],
                        in1=rec[pl:ph, :].unsqueeze(2).broadcast_to([49, HEADS, DH]),
                        op=mybir.AluOpType.mult,
                    )
                st[q] = (vaug, et, o_n)

            def asm(q):
                b, p = q // 4, q % 4
                ot_sb = blkstate[b][2]
                c0 = 98 * p
                nc.vector.tensor_copy(
                    ot_sb[0:128, :, c0 : c0 + 98].rearrange(
                        "p k (w c) -> p k w c", w=2
                    ),
                    avot[0:128, :, 264:362].rearrange("p a (b c) -> p b a c", b=2),
                )

            def stage_b2(q):
                b, p = q // 4, q % 4
                o_n = st.pop(q)[2]
                # deferred assemble of the previous pair: by now its transposes
                # are long done, so this does not head-block the DVE FIFO
                if p > 0:
                    asm(q - 1)
                for par in range(2):
                    pl, ph = 64 * par, 64 * par + 49
                    for kc in range(2):
                        nc.tensor.transpose(
                            avot[0:128, par, 264 + 49 * kc : 313 + 49 * kc],
                            o_n[pl:ph, 4 * kc : 4 * kc + 4, :].rearrange(
                                "p a b -> p (a b)"
                            ),
                            id_sb[pl:ph, :],
                        )
                if p == 3:
                    asm(q)

            def outproj(b):
                off = T * b
                ot_sb = blkstate.pop(b)[2]
                for mc in range(2):
                    pps = pbig.tile([128, 512], f32, tag="big")
                    for kc in range(2):
                        nc.tensor.matmul(
                            pps[:, 0:T],
                            lhsT=wo_sb[:, kc, 128 * mc : 128 * (mc + 1)],
                            rhs=ot_sb[:, kc, :],
                            start=(kc == 0),
                            stop=(kc == 1),
                        )
                    os_sb = outpool.tile([128, T], bf16, tag="outs")
                    if mc == 0:
                        nc.scalar.copy(os_sb[:], pps[:, 0:T])
                    else:
                        nc.vector.tensor_copy(os_sb[:], pps[:, 0:T])
                    nc.sync.dma_start(
                        out=outt_d[128 * mc : 128 * (mc + 1), off : off + T],
                        in_=os_sb[:],
                    )

            # --- flat software-pipelined schedule over all pairs ---
            prologue_dma(0)
            proj_groups(0, [0, 1, 2, 3, 4, 5])
            stage_a(0)
            stage_a(1)
            for q in range(NPAIR):
                b, p = q // 4, q % 4
                if p == 0 and b + 1 < NB:
                    prologue_dma(b + 1)
                stage_b1(q)
                if p in (1, 2) and b + 1 < NB:
                    proj_groups(b + 1, [0, 1, 2] if p == 1 else [3, 4, 5])
                if q + 2 < NPAIR:
                    stage_a(q + 2)
                stage_b2(q)
                if p == 3:
                    outproj(b)
    nc.compile()
    return nc


def _host_prep(w_qkv, w_out, bias_table):
    wqkv = np.asarray(w_qkv, dtype=np.float32).copy()
    wqkv[:, 0:DIM] *= SCALE  # fold softmax scale into q projection
    wqkv_b = np.ascontiguousarray(wqkv).astype(BF16)
    wout_b = np.ascontiguousarray(np.asarray(w_out, dtype=np.float32)).astype(BF16)

    rel = _rel_pos_indices(WIN)  # [i, j]
    bias = np.asarray(bias_table, dtype=np.float32)[rel]  # [i, j, h]
    bt = np.ones((128, 3, 3 * N), dtype=np.float32)
    for h in range(HEADS):
        r, s = _head_rs(h)
        blk = np.exp(bias[:, :, h].T)  # [j, i]
        for par in range(2):
            bt[64 * par : 64 * par + 49, r, 49 * s : 49 * s + 49] = blk
    bias_b = bt.astype(BF16)

    ident = np.zeros((128, N), dtype=np.float32)
    ident[0:49] = np.eye(N, dtype=np.float32)
    ident[64:113] = np.eye(N, dtype=np.float32)
    return wqkv_b, wout_b, bias_b, ident


def make_in_maps(x, w_qkv, w_out, bias_table):
    wqkv_b, wout_b, bias_b, ident = _host_prep(w_qkv, w_out, bias_table)
    xf = np.asarray(x, dtype=np.float32).reshape(W_TOTAL, N, DIM)
    in_maps = []
    for c in range(NCORES):
        xs = xf[c * W_CORE : (c + 1) * W_CORE].reshape(TTOT, DIM)
        xs_t = np.ascontiguousarray(xs.T).astype(BF16)
        in_maps.append(
            {
                "xt": xs_t,
                "wqkv": wqkv_b,
                "wout": wout_b,
                "eb": bias_b,
                "ident": ident,
            }
        )
    return in_maps


def kernel(x, w_qkv, w_out, bias_table):
    if "nc" not in _PROG_CACHE:
        _PROG_CACHE["nc"] = _build_program()
    nc = _PROG_CACHE["nc"]

    from concourse.bass_utils import run_bass_kernel_spmd

    in_maps = make_in_maps(x, w_qkv, w_out, bias_table)
    res = run_bass_kernel_spmd(nc, in_maps, list(range(NCORES)))
    outs = []
    for c in range(NCORES):
        ot = np.asarray(res.results[c]["outt"]).astype(np.float32)  # [256, TTOT]
        outs.append(ot.T.reshape(W_CORE, N, DIM))
    full = np.concatenate(outs, axis=0)
    return full.reshape(16, 16, 16, WIN, WIN, DIM).astype(np.float32)
